# revision 35
# baseline (speedup 1.0000x reference)
"""Trainium2 Bass kernel: GQA attention (B=2, S=1024, dim=2048, 32 Q / 8 KV heads).

Sharding: tensor-parallel over the 8 KV head groups -- core c owns q heads
4c..4c+3 and kv head c (wq/wk/wv column shards, wo row shard).  Every core
reads the full x and produces a partial [T, dim] output (fp16); the host
sums the 8 partials in fp32.  All host-side prep (x transpose, weight
permutation, rope tables) is outside the measured device kernel.

Device-side dataflow per core (v2 -- PE-continuity rewrite):
  x^T (host tile-packed) @ Wqkv -> PSUM [t,384] token-major
  RoPE on DVE (2x320-wide mult + 2x160 add/sub, doubled cos/sin tables)
  PE-transpose q_rot/k_rot to d-major
  scores^T = k^T.T @ q^T, two heads packed via PE row groups
  exp on ACT (scale folded), attn@V with ones column for row sums
  normalize: DVE reciprocal + fp32r PE outer-product broadcast (no DMA)
  out^T partial = attn_out^T.T @ wo_shard -> fp16 DRAM
"""

import os
import sys
import numpy as np
from contextlib import ExitStack

sys.path.insert(0, "/opt/trn_rl_repo")

import concourse.bass as bass
import concourse.tile as tile
from concourse import bacc
from concourse import mybir
from concourse.bass_utils import run_bass_kernel_spmd


B, S, DIM = 2, 1024, 2048
HQ, HKV, D = 32, 8, 64
NCORES = 8
T = B * S
NHC = HQ // NCORES            # 4 q heads per core
QCOLS = NHC * D               # 256
WCOLS = QCOLS + D + D         # 384 (q | k | v)
ROPE_THETA = 10000.0
SCALE = 1.0 / float(np.sqrt(D))

F32 = mybir.dt.float32
F32R = mybir.dt.float32r
F16 = mybir.dt.float16
MUL = mybir.AluOpType.mult
ADD = mybir.AluOpType.add
SUB = mybir.AluOpType.subtract
EXP = mybir.ActivationFunctionType.Exp

NT = T // 128                 # 16 token tiles of 128
NTB = NT // B                 # 8 token tiles per batch


def _build():
    nc = bacc.Bacc(
        "TRN2",
        target_bir_lowering=False,
        debug=False,
        num_devices=NCORES,
    )
    # xt: col = ts4*8192 + kd*512 + tl (kd-major within each 4-tile group:
    # each arriving 512KB chunk unlocks one kd-sweep across 4 token tiles)
    xt_d = nc.dram_tensor("xt", [128, 16 * 2048], F16, kind="ExternalInput").ap()
    # wqkv: col = kd*384 + j
    wqkv_d = nc.dram_tensor("wqkv", [128, 16 * WCOLS], F16, kind="ExternalInput").ap()
    # wo: col = hc*2048 + o
    wo_d = nc.dram_tensor("wo", [128, 2 * DIM], F16, kind="ExternalInput").ap()
    # tab: cos320 at tb*320, sin320 at 2560 + tb*320
    tab_d = nc.dram_tensor("tab", [128, 2 * NTB * 320], F16, kind="ExternalInput").ap()
    id_d = nc.dram_tensor("ident", [128, 128], F16, kind="ExternalInput").ap()
    e2_d = nc.dram_tensor("e2sel", [128, 128], F16, kind="ExternalInput").ap()
    outp = nc.dram_tensor("out", [T, DIM], F16, kind="ExternalOutput").ap()

    with tile.TileContext(nc) as tc, ExitStack() as ctx:
        p = lambda name, bufs, space="SBUF": ctx.enter_context(
            tc.tile_pool(name=name, bufs=bufs, space=space)
        )
        p_const = p("const", 1)
        p_P = p("ropeP", 2)
        p_Q = p("ropeQ", 2)
        p_qr = p("qr", 3)
        p_qT = p("qT", 1)
        p_kT = p("kT", 1)
        p_vp = p("vp", 1)
        p_aoT = p("aoT", 1)
        p_es = p("es", 5)
        p_rcp = p("rcp", 2)
        p_osb = p("osb", 2)
        # PSUM: 4 banks scores, 2 banks attn-accum, 2 banks mix
        p_sc = p("sc", 2, space="PSUM")
        p_at = p("at", 2, space="PSUM")
        p_mix = p("mix", 2, space="PSUM")

        # ---- persistent SBUF tiles -------------------------------------
        ident = p_const.tile([128, 128], F16, tag="ident", name="ident")
        e2f = p_const.tile([128, 128], F16, tag="e2f", name="e2f")
        wqkv_sb = p_const.tile([128, 16 * WCOLS], F16, tag="wqkv", name="wqkv")
        wo_sb = p_const.tile([128, 2 * DIM], F16, tag="wo", name="wo")
        tab_sb = p_const.tile([128, 2 * NTB * 320], F16, tag="tab", name="tab")
        xt_sb = p_const.tile([128, 16 * 2048], F16, tag="xt", name="xt")

        qT = [[p_qT.tile([128, S], F16, tag=f"qT{b}{hp}", name="qT")
               for hp in range(2)] for b in range(B)]
        kT = [p_kT.tile([128, S], F16, tag=f"kT{b}", name="kT") for b in range(B)]
        vp = [[p_vp.tile([128, 65], F16, tag=f"vp{b}{t}", name="vp")
               for t in range(NTB)] for b in range(B)]
        aoT = [[p_aoT.tile([128, S], F16, tag=f"aoT{b}{hp}", name="aoT")
                for hp in range(2)] for b in range(B)]

        # ---- input DMAs ------------------------------------------------
        # Two parallel hardware DMA queues: the sync queue streams the
        # compute-critical wqkv/xt/wo in consumption order; the gpsimd queue
        # carries consts (and later all output writes) so they never delay
        # the critical read stream.
        def dma(dst, src):
            nc.sync.dma_start(dst, src)

        def xt_dma(tt):
            c0 = tt * 2048
            dma(xt_sb[:, c0:c0 + 2048], xt_d[:, c0:c0 + 2048])

        # The first 4-tile group is DMA-bound: interleave wqkv/xt chunks in
        # exactly the kd-sweep order phase A consumes them.
        def xt_dma4(ts4, kdg):
            c0 = ts4 * 8192 + kdg * 2048
            dma(xt_sb[:, c0:c0 + 2048], xt_d[:, c0:c0 + 2048])

        def wqkv_dma(kdg):
            c0 = kdg * 4 * WCOLS
            dma(wqkv_sb[:, c0:c0 + 4 * WCOLS], wqkv_d[:, c0:c0 + 4 * WCOLS])

        nc.gpsimd.dma_start(ident[:], id_d[:])
        nc.gpsimd.dma_start(tab_sb[:], tab_d[:])
        nc.gpsimd.dma_start(e2f[:], e2_d[:])
        for kdg in range(4):
            wqkv_dma(kdg)
            xt_dma4(0, kdg)
        for ts4 in range(1, 4):
            for kdg in range(4):
                xt_dma4(ts4, kdg)
        dma(wo_sb[:], wo_d[:])

        # ones column scaled 1/64 (and v scaled 1/64 to match) so the f16
        # sums row cannot overflow; the normalization ratio is unchanged.
        VSC = 1.0 / 64.0
        for b in range(B):
            for t in range(NTB):
                nc.vector.memset(vp[b][t][:, 64:65], VSC)

        # ---- QKV projection + RoPE + transposes ------------------------
        v5 = lambda ap: ap.rearrange("q (g e i) -> q g e i", g=5, e=2, i=32)

        def qkv_post(tt, ps):
            b, tb = tt // NTB, tt % NTB
            cosv = tab_sb[:, tb * 320:(tb + 1) * 320]
            sinv = tab_sb[:, 2560 + tb * 320:2560 + (tb + 1) * 320]
            X = ps[:, 0:320]
            P = p_P.tile([128, 320], F32, tag="P", name="P")
            Q = p_Q.tile([128, 320], F32, tag="Q", name="Q")
            nc.vector.tensor_tensor(P[:], X, cosv, MUL)
            nc.vector.tensor_tensor(Q[:], X, sinv, MUL)
            qr = p_qr.tile([128, 320], F16, tag="qr", name="qr")
            Pv, Qv, qv = v5(P[:]), v5(Q[:]), v5(qr[:])
            nc.vector.tensor_tensor(qv[:, :, 0, :], Pv[:, :, 0, :],
                                    Qv[:, :, 1, :], SUB)
            nc.vector.tensor_tensor(qv[:, :, 1, :], Qv[:, :, 0, :],
                                    Pv[:, :, 1, :], ADD)
            nc.vector.tensor_scalar_mul(vp[b][tb][:, 0:64], ps[:, 320:384], VSC)
            ps_tr = p_mix.tile([128, 384], F16, tag="mix", name="tr")
            for blk in range(2):
                nc.tensor.transpose(
                    ps_tr[:, blk * 128:(blk + 1) * 128],
                    qr[:, blk * 128:(blk + 1) * 128],
                    ident[:],
                )
            nc.tensor.transpose(ps_tr[0:64, 256:384], qr[:, 256:320], ident[:])
            nc.vector.tensor_copy(qT[b][0][:, tb * 128:(tb + 1) * 128],
                                  ps_tr[:, 0:128])
            nc.scalar.copy(qT[b][1][:, tb * 128:(tb + 1) * 128],
                           ps_tr[:, 128:256])
            nc.scalar.copy(kT[b][0:64, tb * 128:(tb + 1) * 128],
                           ps_tr[0:64, 256:384])
            nc.vector.tensor_copy(kT[b][64:128, tb * 128:(tb + 1) * 128],
                                  ps_tr[0:64, 256:384])

        def qkv_gen(b, psum_slots):
            """Yield-quantized QKV for batch b; psum_slots = list of (pool, tag)."""
            pending = None
            si = 0
            for tb in range(NTB):
                tt = b * NTB + tb
                ts4, ti = tt // 4, tt % 4
                pool, tag = psum_slots[si % len(psum_slots)]
                si += 1
                ps = pool.tile([128, WCOLS], F32, tag=tag, name="qkv")
                for kd in range(16):
                    c = ts4 * 8192 + kd * 512 + ti * 128
                    nc.tensor.matmul(
                        ps[:],
                        xt_sb[:, c:c + 128],
                        wqkv_sb[:, kd * WCOLS:(kd + 1) * WCOLS],
                        start=(kd == 0),
                        stop=(kd == 15),
                    )
                    if kd % 4 == 3:
                        yield
                if pending is not None:
                    qkv_post(*pending)
                    yield
                pending = (tt, ps)
            qkv_post(*pending)
            yield

        def qkv_phaseA():
            # b=0 dense QKV, kd-major across each 4-tile group so the PE
            # consumes chunks in exact DMA arrival order.
            slots = [(p_sc, "sc"), (p_at, "at"), (p_sc, "sc"), (p_at, "at")]
            for g in range(2):
                ps4 = [slots[i][0].tile([128, WCOLS], F32, tag=slots[i][1],
                                        name="qkv") for i in range(4)]
                for kdg in range(4):
                    for ti in range(4):
                        for kd in range(kdg * 4, kdg * 4 + 4):
                            c = g * 8192 + kd * 512 + ti * 128
                            nc.tensor.matmul(
                                ps4[ti][:],
                                xt_sb[:, c:c + 128],
                                wqkv_sb[:, kd * WCOLS:(kd + 1) * WCOLS],
                                start=(kd == 0),
                                stop=(kd == 15),
                            )
                        if kdg == 3:
                            qkv_post(g * 4 + ti, ps4[ti])

        # ---- attention group (2 heads x 512 queries) -------------------
        def attn_group(b, hp, qc, fill1, n_fill=1):
            def filler():
                for _ in range(n_fill):
                    fill1()
            qcol = slice(qc * 512, (qc + 1) * 512)
            ps_at = [p_at.tile([65, 512], F32, tag="at", name="at")
                     for _ in range(2)]
            es_q = {}
            for kc in range(NTB + 1):
                if kc < NTB:
                    ps_s = p_sc.tile([128, 1024], F32, tag="sc", name="sc")
                    for w in range(2):
                        nc.tensor.matmul(
                            ps_s[:, w * 512:(w + 1) * 512],
                            kT[b][w * 64:(w + 1) * 64, kc * 128:(kc + 1) * 128],
                            qT[b][hp][w * 64:(w + 1) * 64, qcol],
                            start=True,
                            stop=True,
                            tile_position=(w * 64, 0),
                        )
                    e = p_es.tile([128, 1024], F16, tag="es", name="es")
                    nc.scalar.activation(e[:], ps_s[:], EXP, scale=SCALE)
                    es_q[kc] = e
                if kc >= 1:
                    e_prev = es_q.pop(kc - 1)
                    for w in range(2):
                        nc.tensor.matmul(
                            ps_at[w][:],
                            vp[b][kc - 1][:],
                            e_prev[:, w * 512:(w + 1) * 512],
                            start=(kc - 1 == 0),
                            stop=(kc - 1 == NTB - 1),
                        )
                filler()
            # normalize: f16 sums rows (partitions 0/32) -> single PE
            # broadcast matmul -> wide reciprocal (psum -> sbuf) -> multiply.
            sums = p_rcp.tile([33, 512], F16, tag="sums", name="sums")
            nc.vector.tensor_copy(sums[0:1, :], ps_at[0][64:65, :])
            nc.vector.tensor_copy(sums[32:33, :], ps_at[1][64:65, :])
            ps_bc = p_mix.tile([128, 512], F32, tag="mix", name="bc")
            nc.tensor.matmul(
                ps_bc[:],
                e2f[0:33, :],
                sums[0:33, :],
                start=True,
                stop=True,
            )
            filler()
            bc_sb = p_rcp.tile([128, 512], F32, tag="bcsb", name="bcsb")
            nc.vector.reciprocal_approx_fast(out=bc_sb[:], in_=ps_bc[:])
            filler()
            for w in range(2):
                nc.vector.tensor_tensor(
                    aoT[b][hp][w * 64:(w + 1) * 64, qcol],
                    ps_at[w][0:64, :],
                    bc_sb[w * 64:(w + 1) * 64, :],
                    MUL,
                )
            filler()

        # ---- output projection -----------------------------------------
        def outproj_gen(b, psum_slots):
            si = 0
            for tb in range(NTB):
                osb = p_osb.tile([128, DIM], F16, tag="osb", name="osb")
                for o in range(4):
                    pool, tag = psum_slots[si % len(psum_slots)]
                    si += 1
                    ps = pool.tile([128, 512], F32, tag=tag, name="op")
                    for hc in range(2):
                        nc.tensor.matmul(
                            ps[:],
                            aoT[b][hc][:, tb * 128:(tb + 1) * 128],
                            wo_sb[:, hc * DIM + o * 512:hc * DIM + (o + 1) * 512],
                            start=(hc == 0),
                            stop=(hc == 1),
                        )
                    yield
                    if o % 2 == 0:
                        nc.vector.tensor_copy(osb[:, o * 512:(o + 1) * 512], ps[:])
                    else:
                        nc.scalar.copy(osb[:, o * 512:(o + 1) * 512], ps[:])
                    if o % 2 == 1:
                        r0 = (b * NTB + tb) * 128
                        c0 = (o - 1) * 512
                        nc.gpsimd.dma_start(
                            outp[r0:r0 + 128, c0:c0 + 1024],
                            osb[:, c0:c0 + 1024],
                        )
                    yield

        def drain(gen):
            for _ in gen:
                pass

        def pump(gen, n):
            for _ in range(n):
                next(gen, None)

        # ---- schedule ---------------------------------------------------
        import itertools
        # Phase A: dense QKV b=0 (psum rotates through idle sc+at banks).
        qkv_phaseA()
        # Phase B: attention b=0 with QKV b=1 as PE filler (mix-bank psum).
        g_qkv1 = qkv_gen(1, [(p_mix, "mix")])
        pump(g_qkv1, 10)
        fill1 = lambda: next(g_qkv1, None)
        for qc in range(2):
            for hp in range(2):
                attn_group(0, hp, qc, fill1, n_fill=2)
        drain(g_qkv1)
        # Phase C: attention b=1 with out-proj b=0 then b=1 as PE filler
        # (b=1 token tiles 0-3 only need the qc=0 groups, which come first).
        op_slots = [(p_mix, "mix")]
        g_op = itertools.chain(
            outproj_gen(0, op_slots),
            outproj_gen(1, op_slots),
        )
        pump(g_op, 3)
        fill0 = lambda: next(g_op, None)
        for qc in range(2):
            for hp in range(2):
                attn_group(1, hp, qc, fill0, n_fill=3)
        # Phase D: remaining out-proj b=1, with sc/at banks now free for
        # deeper psum rotation.
        op_slots.extend([(p_sc, "sc"), (p_at, "at")])
        drain(g_op)
    nc.compile()
    return nc


_CACHE = {}


def _get_program():
    if "nc" not in _CACHE:
        _CACHE["nc"] = _build()
    return _CACHE["nc"]


def host_inputs(x, wq, wk, wv, wo):
    """Host-side prep: tile-pack x/weights, rope tables, per-core shards."""
    import ml_dtypes
    f16 = ml_dtypes.float16 if hasattr(ml_dtypes, "float16") else np.float16
    x = np.asarray(x, dtype=np.float32).reshape(T, DIM)
    # xt: [128, ts4*8192 + kd*512 + tl]
    xT = x.T.astype(f16)                                   # [dim, T]
    xt_pack = np.ascontiguousarray(
        xT.reshape(16, 128, 4, 512).transpose(1, 2, 0, 3).reshape(128, 16 * 2048)
    )
    perm = np.concatenate([np.arange(0, D, 2), np.arange(1, D, 2)])
    inv_freq = 1.0 / (ROPE_THETA ** (np.arange(0, D, 2, dtype=np.float64) / D))
    pos = np.arange(S, dtype=np.float64)
    ang = pos[:, None] * inv_freq[None, :]                 # [S, 32]
    cosb = np.cos(ang).astype(np.float32).reshape(NTB, 128, 1, 1, 32)
    sinb = np.sin(ang).astype(np.float32).reshape(NTB, 128, 1, 1, 32)
    cos320 = np.broadcast_to(cosb, (NTB, 128, 5, 2, 32))
    sin320 = np.broadcast_to(sinb, (NTB, 128, 5, 2, 32))
    tab = np.concatenate(
        [
            cos320.transpose(1, 0, 2, 3, 4).reshape(128, NTB * 320),
            sin320.transpose(1, 0, 2, 3, 4).reshape(128, NTB * 320),
        ],
        axis=1,
    ).astype(f16)
    tab = np.ascontiguousarray(tab)
    ident = np.eye(128, dtype=np.float32).astype(f16)
    e2 = np.zeros((128, 128), dtype=np.float32)
    e2[0, 0:64] = 1.0
    e2[32, 64:128] = 1.0
    e2 = e2.astype(f16)
    wq = np.asarray(wq, dtype=np.float32)
    wk = np.asarray(wk, dtype=np.float32)
    wv = np.asarray(wv, dtype=np.float32)
    wo = np.asarray(wo, dtype=np.float32)
    in_maps = []
    for c in range(NCORES):
        wq_c = wq[:, c * QCOLS:(c + 1) * QCOLS].reshape(DIM, NHC, D)[:, :, perm]
        wq_c = wq_c.reshape(DIM, QCOLS)
        wk_c = wk[:, c * D:(c + 1) * D][:, perm]
        wv_c = wv[:, c * D:(c + 1) * D]
        wqkv_c = np.concatenate([wq_c, wk_c, wv_c], axis=1)      # [2048, 384]
        wqkv_pack = np.ascontiguousarray(
            wqkv_c.reshape(16, 128, WCOLS).transpose(1, 0, 2)
            .reshape(128, 16 * WCOLS).astype(f16)
        )
        wo_c = wo[c * QCOLS:(c + 1) * QCOLS, :]                  # [256, 2048]
        wo_pack = np.ascontiguousarray(
            wo_c.reshape(2, 128, DIM).transpose(1, 0, 2)
            .reshape(128, 2 * DIM).astype(f16)
        )
        in_maps.append(
            {
                "xt": xt_pack,
                "wqkv": wqkv_pack,
                "wo": wo_pack,
                "tab": tab,
                "ident": ident,
                "e2sel": e2,
            }
        )
    return in_maps


def kernel(x, wq, wk, wv, wo):
    nc = _get_program()
    in_maps = host_inputs(x, wq, wk, wv, wo)
    trace = bool(int(os.environ.get("KERNEL_TRACE", "0")))
    import time as _time
    _t0 = _time.time()
    res = run_bass_kernel_spmd(nc, in_maps, list(range(NCORES)), trace=trace)
    _CACHE["run_wall_s"] = _time.time() - _t0
    _CACHE["last_results"] = res
    acc = res.results[0]["out"].astype(np.float32)
    for c in range(1, NCORES):
        acc += res.results[c]["out"].astype(np.float32)
    return acc.reshape(B, S, DIM)


# revision 36
# speedup vs baseline: 1.0268x; 1.0268x over previous
"""Trainium2 Bass kernel: GQA attention (B=2, S=1024, dim=2048, 32 Q / 8 KV heads).

Sharding: tensor-parallel over the 8 KV head groups -- core c owns q heads
4c..4c+3 and kv head c (wq/wk/wv column shards, wo row shard).  Every core
reads the full x and produces a partial [T, dim] output (fp16); the host
sums the 8 partials in fp32.  All host-side prep (x transpose, weight
permutation, rope tables) is outside the measured device kernel.

Device-side dataflow per core (v2 -- PE-continuity rewrite):
  x^T (host tile-packed) @ Wqkv -> PSUM [t,384] token-major
  RoPE on DVE (2x320-wide mult + 2x160 add/sub, doubled cos/sin tables)
  PE-transpose q_rot/k_rot to d-major
  scores^T = k^T.T @ q^T, two heads packed via PE row groups
  exp on ACT (scale folded), attn@V with ones column for row sums
  normalize: DVE reciprocal + fp32r PE outer-product broadcast (no DMA)
  out^T partial = attn_out^T.T @ wo_shard -> fp16 DRAM
"""

import os
import sys
import numpy as np
from contextlib import ExitStack

sys.path.insert(0, "/opt/trn_rl_repo")

import concourse.bass as bass
import concourse.tile as tile
from concourse import bacc
from concourse import mybir
from concourse.bass_utils import run_bass_kernel_spmd


B, S, DIM = 2, 1024, 2048
HQ, HKV, D = 32, 8, 64
NCORES = 8
T = B * S
NHC = HQ // NCORES            # 4 q heads per core
QCOLS = NHC * D               # 256
WCOLS = QCOLS + D + D         # 384 (q | k | v)
ROPE_THETA = 10000.0
SCALE = 1.0 / float(np.sqrt(D))

F32 = mybir.dt.float32
F32R = mybir.dt.float32r
F16 = mybir.dt.float16
MUL = mybir.AluOpType.mult
ADD = mybir.AluOpType.add
SUB = mybir.AluOpType.subtract
EXP = mybir.ActivationFunctionType.Exp

NT = T // 128                 # 16 token tiles of 128
NTB = NT // B                 # 8 token tiles per batch


def _build():
    nc = bacc.Bacc(
        "TRN2",
        target_bir_lowering=False,
        debug=False,
        num_devices=NCORES,
    )
    # xt: col = ts4*8192 + kd*512 + tl (kd-major within each 4-tile group:
    # each arriving 512KB chunk unlocks one kd-sweep across 4 token tiles)
    xt_d = nc.dram_tensor("xt", [128, 16 * 2048], F16, kind="ExternalInput").ap()
    # wqkv: col = kd*384 + j
    wqkv_d = nc.dram_tensor("wqkv", [128, 16 * WCOLS], F16, kind="ExternalInput").ap()
    # wo: col = hc*2048 + o
    wo_d = nc.dram_tensor("wo", [128, 2 * DIM], F16, kind="ExternalInput").ap()
    # tab: cos320 at tb*320, sin320 at 2560 + tb*320
    tab_d = nc.dram_tensor("tab", [128, 2 * NTB * 320], F16, kind="ExternalInput").ap()
    id_d = nc.dram_tensor("ident", [128, 128], F16, kind="ExternalInput").ap()
    e2_d = nc.dram_tensor("e2sel", [128, 128], F16, kind="ExternalInput").ap()
    outp = nc.dram_tensor("out", [T, DIM], F16, kind="ExternalOutput").ap()

    with tile.TileContext(nc) as tc, ExitStack() as ctx:
        p = lambda name, bufs, space="SBUF": ctx.enter_context(
            tc.tile_pool(name=name, bufs=bufs, space=space)
        )
        p_const = p("const", 1)
        p_P = p("ropeP", 2)
        p_Q = p("ropeQ", 2)
        p_qr = p("qr", 3)
        p_qT = p("qT", 1)
        p_kT = p("kT", 1)
        p_vp = p("vp", 1)
        p_aoT = p("aoT", 1)
        p_es = p("es", 5)
        p_rcp = p("rcp", 2)
        p_osb = p("osb", 2)
        # PSUM: 4 banks scores, 2 banks attn-accum, 2 banks mix
        p_sc = p("sc", 2, space="PSUM")
        p_at = p("at", 2, space="PSUM")
        p_mix = p("mix", 2, space="PSUM")

        # ---- persistent SBUF tiles -------------------------------------
        ident = p_const.tile([128, 128], F16, tag="ident", name="ident")
        e2f = p_const.tile([128, 128], F16, tag="e2f", name="e2f")
        wqkv_sb = p_const.tile([128, 16 * WCOLS], F16, tag="wqkv", name="wqkv")
        wo_sb = p_const.tile([128, 2 * DIM], F16, tag="wo", name="wo")
        tab_sb = p_const.tile([128, 2 * NTB * 320], F16, tag="tab", name="tab")
        xt_sb = p_const.tile([128, 16 * 2048], F16, tag="xt", name="xt")

        qT = [[p_qT.tile([128, S], F16, tag=f"qT{b}{hp}", name="qT")
               for hp in range(2)] for b in range(B)]
        kT = [p_kT.tile([128, S], F16, tag=f"kT{b}", name="kT") for b in range(B)]
        vp = [[p_vp.tile([128, 65], F16, tag=f"vp{b}{t}", name="vp")
               for t in range(NTB)] for b in range(B)]
        aoT = [[p_aoT.tile([128, S], F16, tag=f"aoT{b}{hp}", name="aoT")
                for hp in range(2)] for b in range(B)]

        # ---- input DMAs ------------------------------------------------
        # Two parallel hardware DMA queues: the sync queue streams the
        # compute-critical wqkv/xt/wo in consumption order; the gpsimd queue
        # carries consts (and later all output writes) so they never delay
        # the critical read stream.
        def dma(dst, src):
            nc.sync.dma_start(dst, src)

        def xt_dma(tt):
            c0 = tt * 2048
            dma(xt_sb[:, c0:c0 + 2048], xt_d[:, c0:c0 + 2048])

        # The first 4-tile group is DMA-bound: interleave wqkv/xt chunks in
        # exactly the kd-sweep order phase A consumes them.
        def xt_dma4(ts4, kdg):
            c0 = ts4 * 8192 + kdg * 2048
            dma(xt_sb[:, c0:c0 + 2048], xt_d[:, c0:c0 + 2048])

        def wqkv_dma(kdg):
            c0 = kdg * 4 * WCOLS
            dma(wqkv_sb[:, c0:c0 + 4 * WCOLS], wqkv_d[:, c0:c0 + 4 * WCOLS])

        nc.gpsimd.dma_start(ident[:], id_d[:])
        # rope tables split by need order: tiles 0-3's cos/sin first
        nc.gpsimd.dma_start(tab_sb[:, 0:1280], tab_d[:, 0:1280])
        nc.gpsimd.dma_start(tab_sb[:, 2560:3840], tab_d[:, 2560:3840])
        nc.gpsimd.dma_start(e2f[:], e2_d[:])
        nc.gpsimd.dma_start(tab_sb[:, 1280:2560], tab_d[:, 1280:2560])
        nc.gpsimd.dma_start(tab_sb[:, 3840:5120], tab_d[:, 3840:5120])
        for kdg in range(4):
            wqkv_dma(kdg)
            xt_dma4(0, kdg)
        for ts4 in range(1, 4):
            for kdg in range(4):
                xt_dma4(ts4, kdg)
        dma(wo_sb[:], wo_d[:])

        # ones column scaled 1/64 (and v scaled 1/64 to match) so the f16
        # sums row cannot overflow; the normalization ratio is unchanged.
        VSC = 1.0 / 64.0
        for b in range(B):
            for t in range(NTB):
                nc.vector.memset(vp[b][t][:, 64:65], VSC)

        # ---- QKV projection + RoPE + transposes ------------------------
        v5 = lambda ap: ap.rearrange("q (g e i) -> q g e i", g=5, e=2, i=32)

        def qkv_post(tt, ps):
            b, tb = tt // NTB, tt % NTB
            cosv = tab_sb[:, tb * 320:(tb + 1) * 320]
            sinv = tab_sb[:, 2560 + tb * 320:2560 + (tb + 1) * 320]
            X = ps[:, 0:320]
            P = p_P.tile([128, 320], F32, tag="P", name="P")
            Q = p_Q.tile([128, 320], F32, tag="Q", name="Q")
            nc.vector.tensor_tensor(P[:], X, cosv, MUL)
            nc.vector.tensor_tensor(Q[:], X, sinv, MUL)
            qr = p_qr.tile([128, 320], F16, tag="qr", name="qr")
            Pv, Qv, qv = v5(P[:]), v5(Q[:]), v5(qr[:])
            nc.vector.tensor_tensor(qv[:, :, 0, :], Pv[:, :, 0, :],
                                    Qv[:, :, 1, :], SUB)
            nc.vector.tensor_tensor(qv[:, :, 1, :], Qv[:, :, 0, :],
                                    Pv[:, :, 1, :], ADD)
            nc.vector.tensor_scalar_mul(vp[b][tb][:, 0:64], ps[:, 320:384], VSC)
            ps_tr = p_mix.tile([128, 384], F16, tag="mix", name="tr")
            for blk in range(2):
                nc.tensor.transpose(
                    ps_tr[:, blk * 128:(blk + 1) * 128],
                    qr[:, blk * 128:(blk + 1) * 128],
                    ident[:],
                )
            nc.tensor.transpose(ps_tr[0:64, 256:384], qr[:, 256:320], ident[:])
            nc.vector.tensor_copy(qT[b][0][:, tb * 128:(tb + 1) * 128],
                                  ps_tr[:, 0:128])
            nc.scalar.copy(qT[b][1][:, tb * 128:(tb + 1) * 128],
                           ps_tr[:, 128:256])
            nc.scalar.copy(kT[b][0:64, tb * 128:(tb + 1) * 128],
                           ps_tr[0:64, 256:384])
            nc.vector.tensor_copy(kT[b][64:128, tb * 128:(tb + 1) * 128],
                                  ps_tr[0:64, 256:384])

        def qkv_gen(b, psum_slots):
            """Yield-quantized QKV for batch b; psum_slots = list of (pool, tag)."""
            pending = None
            si = 0
            for tb in range(NTB):
                tt = b * NTB + tb
                ts4, ti = tt // 4, tt % 4
                pool, tag = psum_slots[si % len(psum_slots)]
                si += 1
                ps = pool.tile([128, WCOLS], F32, tag=tag, name="qkv")
                for kd in range(16):
                    c = ts4 * 8192 + kd * 512 + ti * 128
                    nc.tensor.matmul(
                        ps[:],
                        xt_sb[:, c:c + 128],
                        wqkv_sb[:, kd * WCOLS:(kd + 1) * WCOLS],
                        start=(kd == 0),
                        stop=(kd == 15),
                    )
                    if kd % 4 == 3:
                        yield
                if pending is not None:
                    qkv_post(*pending)
                    yield
                pending = (tt, ps)
            qkv_post(*pending)
            yield

        def qkv_phaseA():
            # b=0 dense QKV, kd-major across each 4-tile group so the PE
            # consumes chunks in exact DMA arrival order.
            slots = [(p_sc, "sc"), (p_at, "at"), (p_sc, "sc"), (p_at, "at")]
            for g in range(2):
                ps4 = [slots[i][0].tile([128, WCOLS], F32, tag=slots[i][1],
                                        name="qkv") for i in range(4)]
                for kdg in range(4):
                    for ti in range(4):
                        for kd in range(kdg * 4, kdg * 4 + 4):
                            c = g * 8192 + kd * 512 + ti * 128
                            nc.tensor.matmul(
                                ps4[ti][:],
                                xt_sb[:, c:c + 128],
                                wqkv_sb[:, kd * WCOLS:(kd + 1) * WCOLS],
                                start=(kd == 0),
                                stop=(kd == 15),
                            )
                        if kdg == 3:
                            qkv_post(g * 4 + ti, ps4[ti])

        # ---- attention group (2 heads x 512 queries) -------------------
        def attn_group(b, hp, qc, fill1, n_fill=1):
            def filler():
                for _ in range(n_fill):
                    fill1()
            qcol = slice(qc * 512, (qc + 1) * 512)
            ps_at = [p_at.tile([65, 512], F32, tag="at", name="at")
                     for _ in range(2)]
            es_q = {}
            for kc in range(NTB + 1):
                if kc < NTB:
                    ps_s = p_sc.tile([128, 1024], F32, tag="sc", name="sc")
                    for w in range(2):
                        nc.tensor.matmul(
                            ps_s[:, w * 512:(w + 1) * 512],
                            kT[b][w * 64:(w + 1) * 64, kc * 128:(kc + 1) * 128],
                            qT[b][hp][w * 64:(w + 1) * 64, qcol],
                            start=True,
                            stop=True,
                            tile_position=(w * 64, 0),
                        )
                    e = p_es.tile([128, 1024], F16, tag="es", name="es")
                    nc.scalar.activation(e[:], ps_s[:], EXP, scale=SCALE)
                    es_q[kc] = e
                if kc >= 1:
                    e_prev = es_q.pop(kc - 1)
                    for w in range(2):
                        nc.tensor.matmul(
                            ps_at[w][:],
                            vp[b][kc - 1][:],
                            e_prev[:, w * 512:(w + 1) * 512],
                            start=(kc - 1 == 0),
                            stop=(kc - 1 == NTB - 1),
                        )
                filler()
            # normalize: f16 sums rows (partitions 0/32) -> single PE
            # broadcast matmul -> wide reciprocal (psum -> sbuf) -> multiply.
            sums = p_rcp.tile([33, 512], F16, tag="sums", name="sums")
            nc.vector.tensor_copy(sums[0:1, :], ps_at[0][64:65, :])
            nc.vector.tensor_copy(sums[32:33, :], ps_at[1][64:65, :])
            ps_bc = p_mix.tile([128, 512], F32, tag="mix", name="bc")
            nc.tensor.matmul(
                ps_bc[:],
                e2f[0:33, :],
                sums[0:33, :],
                start=True,
                stop=True,
            )
            filler()
            bc_sb = p_rcp.tile([128, 512], F32, tag="bcsb", name="bcsb")
            nc.vector.reciprocal_approx_fast(out=bc_sb[:], in_=ps_bc[:])
            filler()
            for w in range(2):
                nc.vector.tensor_tensor(
                    aoT[b][hp][w * 64:(w + 1) * 64, qcol],
                    ps_at[w][0:64, :],
                    bc_sb[w * 64:(w + 1) * 64, :],
                    MUL,
                )
            filler()

        # ---- output projection -----------------------------------------
        def outproj_gen(b, psum_slots):
            si = 0
            for tb in range(NTB):
                osb = p_osb.tile([128, DIM], F16, tag="osb", name="osb")
                for o in range(4):
                    pool, tag = psum_slots[si % len(psum_slots)]
                    si += 1
                    ps = pool.tile([128, 512], F32, tag=tag, name="op")
                    for hc in range(2):
                        nc.tensor.matmul(
                            ps[:],
                            aoT[b][hc][:, tb * 128:(tb + 1) * 128],
                            wo_sb[:, hc * DIM + o * 512:hc * DIM + (o + 1) * 512],
                            start=(hc == 0),
                            stop=(hc == 1),
                        )
                    yield
                    if o % 2 == 0:
                        nc.vector.tensor_copy(osb[:, o * 512:(o + 1) * 512], ps[:])
                    else:
                        nc.scalar.copy(osb[:, o * 512:(o + 1) * 512], ps[:])
                    if o % 2 == 1:
                        r0 = (b * NTB + tb) * 128
                        c0 = (o - 1) * 512
                        nc.gpsimd.dma_start(
                            outp[r0:r0 + 128, c0:c0 + 1024],
                            osb[:, c0:c0 + 1024],
                        )
                    yield

        def drain(gen):
            for _ in gen:
                pass

        def pump(gen, n):
            for _ in range(n):
                next(gen, None)

        # ---- schedule ---------------------------------------------------
        import itertools
        # Phase A: dense QKV b=0 (psum rotates through idle sc+at banks).
        qkv_phaseA()
        # Phase B: attention b=0 with QKV b=1 as PE filler (mix-bank psum).
        g_qkv1 = qkv_gen(1, [(p_mix, "mix")])
        pump(g_qkv1, 10)
        fill1 = lambda: next(g_qkv1, None)
        for qc in range(2):
            for hp in range(2):
                attn_group(0, hp, qc, fill1, n_fill=2)
        drain(g_qkv1)
        # Phase C: attention b=1 with out-proj b=0 then b=1 as PE filler
        # (b=1 token tiles 0-3 only need the qc=0 groups, which come first).
        op_slots = [(p_mix, "mix")]
        g_op = itertools.chain(
            outproj_gen(0, op_slots),
            outproj_gen(1, op_slots),
        )
        pump(g_op, 3)
        fill0 = lambda: next(g_op, None)
        for qc in range(2):
            for hp in range(2):
                attn_group(1, hp, qc, fill0, n_fill=3)
        # Phase D: remaining out-proj b=1, with sc/at banks now free for
        # deeper psum rotation.
        op_slots.extend([(p_sc, "sc"), (p_at, "at")])
        drain(g_op)
    nc.compile()
    return nc


_CACHE = {}


def _get_program():
    if "nc" not in _CACHE:
        _CACHE["nc"] = _build()
    return _CACHE["nc"]


def host_inputs(x, wq, wk, wv, wo):
    """Host-side prep: tile-pack x/weights, rope tables, per-core shards."""
    import ml_dtypes
    f16 = ml_dtypes.float16 if hasattr(ml_dtypes, "float16") else np.float16
    x = np.asarray(x, dtype=np.float32).reshape(T, DIM)
    # xt: [128, ts4*8192 + kd*512 + tl]
    xT = x.T.astype(f16)                                   # [dim, T]
    xt_pack = np.ascontiguousarray(
        xT.reshape(16, 128, 4, 512).transpose(1, 2, 0, 3).reshape(128, 16 * 2048)
    )
    perm = np.concatenate([np.arange(0, D, 2), np.arange(1, D, 2)])
    inv_freq = 1.0 / (ROPE_THETA ** (np.arange(0, D, 2, dtype=np.float64) / D))
    pos = np.arange(S, dtype=np.float64)
    ang = pos[:, None] * inv_freq[None, :]                 # [S, 32]
    cosb = np.cos(ang).astype(np.float32).reshape(NTB, 128, 1, 1, 32)
    sinb = np.sin(ang).astype(np.float32).reshape(NTB, 128, 1, 1, 32)
    cos320 = np.broadcast_to(cosb, (NTB, 128, 5, 2, 32))
    sin320 = np.broadcast_to(sinb, (NTB, 128, 5, 2, 32))
    tab = np.concatenate(
        [
            cos320.transpose(1, 0, 2, 3, 4).reshape(128, NTB * 320),
            sin320.transpose(1, 0, 2, 3, 4).reshape(128, NTB * 320),
        ],
        axis=1,
    ).astype(f16)
    tab = np.ascontiguousarray(tab)
    ident = np.eye(128, dtype=np.float32).astype(f16)
    e2 = np.zeros((128, 128), dtype=np.float32)
    e2[0, 0:64] = 1.0
    e2[32, 64:128] = 1.0
    e2 = e2.astype(f16)
    wq = np.asarray(wq, dtype=np.float32)
    wk = np.asarray(wk, dtype=np.float32)
    wv = np.asarray(wv, dtype=np.float32)
    wo = np.asarray(wo, dtype=np.float32)
    in_maps = []
    for c in range(NCORES):
        wq_c = wq[:, c * QCOLS:(c + 1) * QCOLS].reshape(DIM, NHC, D)[:, :, perm]
        wq_c = wq_c.reshape(DIM, QCOLS)
        wk_c = wk[:, c * D:(c + 1) * D][:, perm]
        wv_c = wv[:, c * D:(c + 1) * D]
        wqkv_c = np.concatenate([wq_c, wk_c, wv_c], axis=1)      # [2048, 384]
        wqkv_pack = np.ascontiguousarray(
            wqkv_c.reshape(16, 128, WCOLS).transpose(1, 0, 2)
            .reshape(128, 16 * WCOLS).astype(f16)
        )
        wo_c = wo[c * QCOLS:(c + 1) * QCOLS, :]                  # [256, 2048]
        wo_pack = np.ascontiguousarray(
            wo_c.reshape(2, 128, DIM).transpose(1, 0, 2)
            .reshape(128, 2 * DIM).astype(f16)
        )
        in_maps.append(
            {
                "xt": xt_pack,
                "wqkv": wqkv_pack,
                "wo": wo_pack,
                "tab": tab,
                "ident": ident,
                "e2sel": e2,
            }
        )
    return in_maps


def kernel(x, wq, wk, wv, wo):
    nc = _get_program()
    in_maps = host_inputs(x, wq, wk, wv, wo)
    trace = bool(int(os.environ.get("KERNEL_TRACE", "0")))
    import time as _time
    _t0 = _time.time()
    res = run_bass_kernel_spmd(nc, in_maps, list(range(NCORES)), trace=trace)
    _CACHE["run_wall_s"] = _time.time() - _t0
    _CACHE["last_results"] = res
    acc = res.results[0]["out"].astype(np.float32)
    for c in range(1, NCORES):
        acc += res.results[c]["out"].astype(np.float32)
    return acc.reshape(B, S, DIM)
